# revision 4
# baseline (speedup 1.0000x reference)
"""BiGRU Trainium2 kernel, 8-core SPMD.

Strategy: shard the hidden dimension H=1024 8 ways (128 per core). Each core
computes its 128-wide slice of both GRU directions for the full batch; the
per-step hidden state is exchanged between all cores with SWDGE remote DMA
(SBUF -> SBUF, one receive slot per peer). The recurrence matmul is
hidden-state-stationary: lhsT = h^T tiles (K = H on partitions), rhs = Whh^T
column slices, so the PE streams weight columns at full rate; fwd and bwd
directions run concurrently on the two halves of the PE array (out partition
base 0 / 64).

The input projection xg = x @ Wih^T + biases (both directions) is computed
on-device, interleaved with the recurrence to fill PE idle time, and staged
through DRAM in [t*B + b] row order so each step loads contiguous tiles.

SPMD twist: remote-DMA relative destinations XOR the *physical* NC index and
instruction streams are identical on all cores, so per-core differences live
in data only. Receive slot d on logical core r holds the h-slice of core
sigma_r(d) = FINV[F[r] ^ d] (F = logical->physical NC map); the host permutes
each core's Whh^T / W_lin^T contraction blocks by sigma_r so one static slot
order is correct everywhere.
"""

import os
import sys

sys.path.insert(0, "/opt/trn_rl_repo")

import numpy as np
import ml_dtypes

import concourse.bass as bass
import concourse.mybir as mybir

# ---- problem constants -------------------------------------------------------
B = 64  # batch
T = 512  # sequence length
I = 1024  # input features
H = 1024  # hidden
O = 1024  # output features
N = 8  # cores
KT = 8  # 128-row contraction blocks in H (and I)
P = 128
SL = 128  # per-core H slice
G3 = 3 * SL  # per-core gate columns (r|z|n)

# logical -> physical NC map of this fabric (measured); relative XOR remote-DMA
# routing operates in physical space.
F_MAP = [0, 1, 2, 3, 6, 7, 4, 5]
FINV = [F_MAP.index(i) for i in range(8)]

BF16 = mybir.dt.bfloat16
F32 = mybir.dt.float32
AFT = mybir.ActivationFunctionType


def sigma(r: int, d: int) -> int:
    """H-slice owner whose tile lands in receive slot d on logical core r."""
    return FINV[F_MAP[r] ^ d]


# ---- device program ----------------------------------------------------------


def build_program(t_steps: int = T):
    """One SPMD Bacc program, identical for all 8 cores.

    t_steps must be even; the phase-1 token tiling assumes
    n_tok_tiles = t_steps / 2 (each tile = 2 t-values x 64 batch rows).
    """
    from concourse.bacc import Bacc

    assert t_steps % 2 == 0
    NTT = t_steps // 2  # phase-1 token tiles
    PRO = min(4, NTT)  # tiles processed before step 0
    XPF = 4  # xg prefetch depth (ring)

    DBG = os.environ.get("KDBG", "").split(",")
    no_bcast = "nobcast" in DBG
    no_epi = "noepi" in DBG
    no_rec = "norec" in DBG
    no_gates = "nogates" in DBG
    no_transp = "notransp" in DBG
    no_mmrec = "nommrec" in DBG
    act_only = "actonly" in DBG
    no_tanh = "notanh" in DBG
    no_dvemix = "nodvemix" in DBG

    nc = Bacc()

    # -- IO -------------------------------------------------------------------
    xT = nc.declare_dram_parameter("xT", [NTT, P, KT * P], BF16, isOutput=False)
    wih = nc.declare_dram_parameter("wih", [KT, P, 2 * G3], BF16, isOutput=False)
    whh = nc.declare_dram_parameter("whh", [KT, P, 2 * G3], BF16, isOutput=False)
    wlin = nc.declare_dram_parameter("wlin", [2 * KT, P, SL], BF16, isOutput=False)
    bias1 = nc.declare_dram_parameter("bias1", [1, 2 * G3], BF16, isOutput=False)
    biasn = nc.declare_dram_parameter("biasn", [1, 2 * SL], BF16, isOutput=False)
    blin = nc.declare_dram_parameter("blin", [1, SL], BF16, isOutput=False)
    ident = nc.declare_dram_parameter("ident", [P, P], BF16, isOutput=False)
    ones = nc.declare_dram_parameter("ones", [1, P], BF16, isOutput=False)
    out = nc.declare_dram_parameter("out", [B, SL], F32, isOutput=True)

    # phase-1 output staging through DRAM, [t*64 + b, 384] row order
    xgf_d = nc.dram_tensor("xgf_d", [t_steps * B, G3], BF16)
    xgb_d = nc.dram_tensor("xgb_d", [t_steps * B, G3], BF16)

    n_init_dma = KT + KT + 2 * KT + 5  # whh, wih, wlin blocks + 5 small consts

    def ph1_tile(p: int) -> int:
        """phase-1 processing order: ends inward (0, NTT-1, 1, NTT-2, ...)."""
        return p // 2 if p % 2 == 0 else NTT - 1 - p // 2

    from contextlib import ExitStack

    es = ExitStack()
    with es:
        sem = lambda name: es.enter_context(nc.semaphore(name))
        sbuf = lambda name, shape, dt=BF16: es.enter_context(
            nc.sbuf_tensor(name, shape, dt)
        )
        psum = lambda name, shape, dt: es.enter_context(nc.psum_tensor(name, shape, dt))

        block = es.enter_context(nc.Block())
        init_sem = sem("init_sem")
        hz_sem = sem("hz_sem")
        bar_sem = sem("bar_sem")
        bar_p = sem("bar_p")
        bar_l = sem("bar_l")
        rsem = [[sem(f"rsem{par}_{d}") for d in range(N)] for par in range(2)]
        lsem = [sem("lsem0"), sem("lsem1")]
        prep_sem = sem("prep_sem")
        psum_rdy = sem("psum_rdy")
        a2v_r = sem("a2v_r")
        a2v_z = sem("a2v_z")
        a2v_n = sem("a2v_n")
        v2a_np = sem("v2a_np")
        pf_v = sem("pf_v")
        v2p = sem("v2p")
        vch = sem("vch")
        p2v = sem("p2v")
        tdone = sem("tdone")
        xg_dma = [sem(f"xg_dma{i}") for i in range(XPF)]
        xgc_p = sem("xgc_p")
        xt_dma = [sem("xt_dma0"), sem("xt_dma1")]
        p1_rdy = sem("p1_rdy")
        p1_cp = sem("p1_cp")
        p1_w = [sem("p1_w0"), sem("p1_w1")]
        fin_sem = sem("fin_sem")

        whh_s = sbuf("whh_s", [P, KT * 2 * G3])
        wih_s = sbuf("wih_s", [P, KT * 2 * G3])
        wlin_s = sbuf("wlin_s", [P, 2 * KT * SL])
        hbuf = sbuf("hbuf", [P, 2 * N * P])
        xg_s = sbuf("xg_s", [P, XPF * G3])
        xt_s = sbuf("xt_s", [P, 2 * KT * P])
        rz_s = sbuf("rz_s", [P, 2 * SL])
        t1_s = sbuf("t1_s", [P, SL])
        npre_s = sbuf("npre_s", [P, SL])
        n_s = sbuf("n_s", [P, SL])
        s1_s = sbuf("s1_s", [P, SL])
        s2_s = sbuf("s2_s", [P, SL])
        hst_s = sbuf("hst_s", [P, SL])
        hgn_s = sbuf("hgn_s", [P, SL])
        tb_s = sbuf("tb_s", [P, 2 * P])
        xgf_st = sbuf("xgf_st", [P, 2 * G3])
        xgb_st = sbuf("xgb_st", [P, 2 * G3])
        ident_s = sbuf("ident_s", [P, P])
        ones_s = sbuf("ones_s", [1, P])
        bias1_s = sbuf("bias1_s", [1, 2 * G3])
        biasn_s = sbuf("biasn_s", [1, 2 * SL])
        blin_s = sbuf("blin_s", [1, SL])
        out_s = sbuf("out_s", [B, SL], F32)
        # separate tensors so double-buffers land in different PSUM banks
        # (PE-write + DVE-read of one bank is a hardware fault)
        ps_rec0 = psum("ps_rec0", [P, G3], F32)
        ps_rec1 = psum("ps_rec1", [P, G3], F32)
        ps_t0 = psum("ps_t0", [P, P], BF16)
        ps_t1 = psum("ps_t1", [P, P], BF16)
        ps_p1f = psum("ps_p1f", [P, G3], F32)
        ps_p1b = psum("ps_p1b", [P, G3], F32)
        ps_rec = [ps_rec0, ps_rec1]
        ps_t = [ps_t0, ps_t1]

        def hb(t):
            """hbuf column offset of the buffer read at step t."""
            return (t % 2) * N * P

        # ---------------- SYNC: all HWDGE DMA traffic ---------------------
        @block.sync
        def _(s):
            for k in range(KT):
                s.dma_start(
                    out=whh_s[:, k * 2 * G3 : (k + 1) * 2 * G3], in_=whh[k, :, :]
                ).then_inc(init_sem, 16)
                s.dma_start(
                    out=wih_s[:, k * 2 * G3 : (k + 1) * 2 * G3], in_=wih[k, :, :]
                ).then_inc(init_sem, 16)
            for k in range(2 * KT):
                s.dma_start(
                    out=wlin_s[:, k * SL : (k + 1) * SL], in_=wlin[k, :, :]
                ).then_inc(init_sem, 16)
            s.dma_start(out=ident_s[:, :], in_=ident[:, :]).then_inc(init_sem, 16)
            s.dma_start(out=ones_s[:, :], in_=ones[:, :]).then_inc(init_sem, 16)
            s.dma_start(out=bias1_s[:, :], in_=bias1[:, :]).then_inc(init_sem, 16)
            s.dma_start(out=biasn_s[:, :], in_=biasn[:, :]).then_inc(init_sem, 16)
            s.dma_start(out=blin_s[:, :], in_=blin[:, :]).then_inc(init_sem, 16)

            def load_xt(p):
                if p >= NTT:
                    return
                if p >= 2:
                    s.wait_ge(p1_rdy, p - 1)  # xt ring slot free
                s.dma_start(
                    out=xt_s[:, (p % 2) * KT * P : ((p % 2) + 1) * KT * P],
                    in_=xT[ph1_tile(p), :, :],
                ).then_inc(xt_dma[p % 2], 16)

            def write_ph1(p):
                if p >= NTT:
                    return
                tau = ph1_tile(p)
                s.wait_ge(p1_cp, 2 * (p + 1))
                s.dma_start(
                    out=xgf_d[2 * tau * B : 2 * tau * B + P, :],
                    in_=xgf_st[:, (p % 2) * G3 : (p % 2) * G3 + G3],
                ).then_inc(p1_w[p % 2], 16)
                s.dma_start(
                    out=xgb_d[2 * tau * B : 2 * tau * B + P, :],
                    in_=xgb_st[:, (p % 2) * G3 : (p % 2) * G3 + G3],
                ).then_inc(p1_w[p % 2], 16)

            def load_xg(t):
                if no_rec or t >= t_steps or t < 0:
                    return
                if load_xg.done >= t + 1:
                    return
                load_xg.done = t + 1
                # phase-1 tiles 0..need_p-1 cover fwd row t and bwd row T-1-t
                need_p = min(2 * (t // 2) + 2, NTT)
                s.wait_ge(p1_w[0], 32 * (need_p - need_p // 2))
                s.wait_ge(p1_w[1], 32 * (need_p // 2))
                if t >= XPF:
                    s.wait_ge(v2a_np, t - XPF + 1)
                    s.wait_ge(xgc_p, t - XPF + 1)
                slot = (t % XPF) * G3
                s.dma_start(
                    out=xg_s[0:B, slot : slot + G3],
                    in_=xgf_d[t * B : (t + 1) * B, :],
                ).then_inc(xg_dma[t % XPF], 16)
                s.dma_start(
                    out=xg_s[B:P, slot : slot + G3],
                    in_=xgb_d[(t_steps - 1 - t) * B : (t_steps - t) * B, :],
                ).then_inc(xg_dma[t % XPF], 16)

            # prologue: interleave so no FIFO head-of-line cycle forms
            # (load_xt(p+2) transitively needs write_ph1(p-2) through PE/DVE)
            for p in range(4):
                load_xt(p)
            write_ph1(0)
            load_xt(4)
            write_ph1(1)
            load_xt(5)
            write_ph1(2)
            write_ph1(3)
            load_xg.done = 0
            for t in range(XPF):
                load_xg(t)
            for t in range(t_steps):
                write_ph1(PRO + t)
                load_xt(PRO + t + 2)
                load_xg(t + XPF - 1)

            s.wait_ge(fin_sem, 1)
            s.dma_start(out=out[:, :], in_=out_s[:, :]).then_inc(fin_sem, 16)

        # ---------------- PE: matmuls, transpose, phase-1 ------------------
        @block.tensor
        def _(pe):
            def ph1_work(p):
                if p >= NTT:
                    return
                pe.wait_ge(xt_dma[p % 2], 16 * (p // 2 + 1))
                if p >= 1:
                    pe.wait_ge(p1_cp, 2 * p)  # psum consumed by DVE copies
                xo = (p % 2) * KT * P
                for k in range(KT):
                    lt = xt_s[:, xo + k * P : xo + (k + 1) * P]
                    pe.matmul(
                        ps_p1f[:, :],
                        lt,
                        wih_s[:, k * 2 * G3 : k * 2 * G3 + G3],
                        start=(k == 0),
                        stop=False,
                    )
                    pe.matmul(
                        ps_p1b[:, :],
                        lt,
                        wih_s[:, k * 2 * G3 + G3 : (k + 1) * 2 * G3],
                        start=(k == 0),
                        stop=False,
                    )
                pe.matmul(
                    ps_p1f[:, :],
                    ones_s[0:1, :],
                    bias1_s[0:1, 0:G3],
                    start=False,
                    stop=True,
                )
                pe.matmul(
                    ps_p1b[:, :],
                    ones_s[0:1, :],
                    bias1_s[0:1, G3 : 2 * G3],
                    start=False,
                    stop=True,
                ).then_inc(p1_rdy, 1)

            pe.wait_ge(init_sem, 16 * n_init_dma)
            pe.wait_ge(hz_sem, 2)
            for p in range(PRO):
                ph1_work(p)

            for t in range(t_steps):
                ps = ps_rec[t % 2]
                if no_rec:
                    ph1_work(PRO + t)
                    continue
                if t >= 1 and not no_bcast:
                    for d in range(N):
                        pe.wait_ge(rsem[(t - 1) % 2][d], 2 * ((t - 1) // 2 + 1))
                if t >= 2:
                    pe.wait_ge(a2v_z, 2 * (t - 1))
                    pe.wait_ge(pf_v, t - 1)
                pe.wait_ge(xg_dma[t % XPF], 32 * (t // XPF + 1))
                hbo = hb(t)
                slot = (t % XPF) * G3
                if no_mmrec:
                    pe.matmul(
                        ps[:, 0 : 2 * SL],
                        ident_s[:, :],
                        xg_s[:, slot : slot + 2 * SL],
                        start=True,
                        stop=True,
                    ).then_inc(xgc_p, 1)
                    pe.matmul(
                        ps[0:B, 2 * SL : G3],
                        ones_s[0:1, 0:B],
                        biasn_s[0:1, 0:SL],
                        start=True,
                        stop=True,
                    )
                    pe.matmul(
                        ps[B:P, 2 * SL : G3],
                        ones_s[0:1, B:P],
                        biasn_s[0:1, SL : 2 * SL],
                        start=True,
                        stop=True,
                        skip_group_check=True,
                    ).then_inc(psum_rdy, 1)
                if not no_mmrec:
                    # d=0 opens the accumulation (start marks the whole bank
                    # pending-zero per partition half); d=7 closes it; the xg and
                    # bhh_n matmuls then accumulate on top (group-check skipped:
                    # their region is a slice of the already-opened groups).
                    for d in range(N):
                        pe.matmul(
                            ps[0:B, :],
                            hbuf[:, hbo + d * P : hbo + d * P + B],
                            whh_s[:, d * 2 * G3 : d * 2 * G3 + G3],
                            start=(d == 0),
                            stop=(d == N - 1),
                        )
                        pe.matmul(
                            ps[B:P, :],
                            hbuf[:, hbo + d * P + B : hbo + (d + 1) * P],
                            whh_s[:, d * 2 * G3 + G3 : (d + 1) * 2 * G3],
                            start=(d == 0),
                            stop=(d == N - 1),
                            skip_group_check=True,
                        )
                    pe.matmul(
                        ps[:, 0 : 2 * SL],
                        ident_s[:, :],
                        xg_s[:, slot : slot + 2 * SL],
                        start=False,
                        stop=False,
                        skip_group_check=True,
                    ).then_inc(xgc_p, 1)
                    pe.matmul(
                        ps[0:B, 2 * SL : G3],
                        ones_s[0:1, 0:B],
                        biasn_s[0:1, 0:SL],
                        start=False,
                        stop=False,
                        skip_group_check=True,
                    )
                    pe.matmul(
                        ps[B:P, 2 * SL : G3],
                        ones_s[0:1, B:P],
                        biasn_s[0:1, SL : 2 * SL],
                        start=False,
                        stop=False,
                        skip_group_check=True,
                    ).then_inc(psum_rdy, 1)

                # phase-1 fill while the gates run on ACT/DVE
                ph1_work(PRO + t)

                # transpose h_new into the broadcast source layout
                if not no_transp:
                    pe.wait_ge(v2p, t + 1)
                    pe.transpose(ps_t[t % 2][:, :], hst_s[:, :], ident_s[:, :]).then_inc(
                        p2v, 1
                    )

            if no_epi or no_rec:
                pe.wait_ge(p1_cp, 2 * NTT)
                pe.matmul(
                    ps_p1f[0:B, 0:SL],
                    ident_s[:, 0:B],
                    blin_s[0:1, :] if False else wlin_s[0:128, 0:SL],
                    start=True,
                    stop=True,
                ).then_inc(psum_rdy, 1 if no_rec else t_steps + 1)
                return

            # final linear: out = [h_fwd | h_bwd] @ W_lin^T + b_lin
            if not no_bcast:
                for d in range(N):
                    pe.wait_ge(
                        rsem[(t_steps - 1) % 2][d], 2 * ((t_steps - 1) // 2 + 1)
                    )
            pe.wait_ge(p1_cp, 2 * NTT)  # ps_p1f free
            hbo = hb(t_steps)
            for d in range(N):
                pe.matmul(
                    ps_p1f[0:B, 0:SL],
                    hbuf[:, hbo + d * P : hbo + d * P + B],
                    wlin_s[:, d * SL : (d + 1) * SL],
                    start=(d == 0),
                    stop=False,
                )
            for d in range(N):
                pe.matmul(
                    ps_p1f[0:B, 0:SL],
                    hbuf[:, hbo + d * P + B : hbo + (d + 1) * P],
                    wlin_s[:, (N + d) * SL : (N + d + 1) * SL],
                    start=False,
                    stop=False,
                )
            pe.matmul(
                ps_p1f[0:B, 0:SL],
                ones_s[0:1, 0:B],
                blin_s[0:1, :],
                start=False,
                stop=True,
            ).then_inc(psum_rdy, 1)

        # ---------------- ACT: sigmoids + tanh ----------------------------
        @block.scalar
        def _(a):
            if no_rec or no_gates:
                return
            for t in range(t_steps):
                ps = ps_rec[t % 2]
                a.wait_ge(psum_rdy, t + 1)
                a.activation(rz_s[:, 0:SL], ps[:, 0:SL], AFT.Sigmoid).then_inc(
                    a2v_r, 1
                )
                a.activation(
                    rz_s[:, SL : 2 * SL], ps[:, SL : 2 * SL], AFT.Sigmoid
                ).then_inc(a2v_z, 1)
                a.activation(hgn_s[:, :], ps[:, 2 * SL : G3], AFT.Copy).then_inc(
                    a2v_z, 1
                )
                if act_only or no_tanh:
                    a.activation(n_s[:, :], npre_s[:, :], AFT.Sigmoid).then_inc(
                        a2v_n, 1
                    )
                else:
                    a.wait_ge(v2a_np, t + 1)
                    a.activation(n_s[:, :], npre_s[:, :], AFT.Tanh).then_inc(a2v_n, 1)

        # ---------------- DVE: gate arithmetic, copies --------------------
        @block.vector
        def _(v):
            v.memset(hbuf[:, :], 0.0).then_inc(hz_sem, 1)
            v.memset(hst_s[:, :], 0.0).then_inc(hz_sem, 1)
            v.wait_ge(hz_sem, 2)

            def ph1_copy(p):
                if p >= NTT:
                    return
                v.wait_ge(p1_rdy, p + 1)
                if p >= 2:
                    v.wait_ge(p1_w[p % 2], 32 * (p // 2))  # staging slot free
                v.tensor_copy(
                    xgf_st[:, (p % 2) * G3 : (p % 2) * G3 + G3], ps_p1f[:, :]
                )
                v.tensor_copy(
                    xgb_st[:, (p % 2) * G3 : (p % 2) * G3 + G3], ps_p1b[:, :]
                ).then_inc(p1_cp, 2)

            for p in range(PRO):
                ph1_copy(p)

            for t in range(t_steps):
                if no_rec:
                    ph1_copy(PRO + t)
                    continue
                ps = ps_rec[t % 2]
                slot = (t % XPF) * G3
                if no_gates:
                    v.wait_ge(psum_rdy, t + 1)
                    v.tensor_copy(s1_s[:, :], hst_s[:, :]).then_inc(v2p, 1)
                    if not no_transp:
                        v.wait_ge(p2v, t + 1)
                        v.tensor_copy(
                            tb_s[:, (t % 2) * P : (t % 2) * P + P], ps_t[t % 2][:, :]
                        ).then_inc(tdone, 1)
                    ph1_copy(PRO + t)
                    continue
                v.wait_ge(a2v_r, t + 1)
                if act_only:
                    v.tensor_copy(s1_s[:, :], hst_s[:, :]).then_inc(v2p, 1)
                    if not no_transp:
                        v.wait_ge(p2v, t + 1)
                        v.tensor_copy(
                            tb_s[:, (t % 2) * P : (t % 2) * P + P], ps_t[t % 2][:, :]
                        ).then_inc(tdone, 1)
                    ph1_copy(PRO + t)
                    continue
                # t1 = r * hg_n (hg_n staged through SBUF by ACT: a DVE
                # TensorTensor read of PSUM hard-faults this device)
                v.wait_ge(a2v_z, 2 * t + 2)
                v.tensor_mul(t1_s[:, :], rz_s[:, 0:SL], hgn_s[:, :]).then_inc(
                    pf_v, 1
                )
                # n_pre = t1 + xg_n
                v.wait_ge(pf_v, t + 1)  # t1 writeback drained
                v.wait_ge(xg_dma[t % XPF], 32 * (t // XPF + 1))
                v.tensor_add(
                    npre_s[:, :], t1_s[:, :], xg_s[:, slot + 2 * SL : slot + G3]
                ).then_inc(v2a_np, 1)
                v.wait_ge(a2v_n, t + 1)
                # h_new = n + z*(h - n)
                if t >= 1:
                    v.wait_ge(v2p, t)  # prior h_new writeback drained
                v.tensor_sub(s1_s[:, :], hst_s[:, :], n_s[:, :]).then_inc(vch, 1)
                v.wait_ge(a2v_z, 2 * t + 1)
                v.wait_ge(vch, 2 * t + 1)
                v.tensor_mul(s2_s[:, :], rz_s[:, SL : 2 * SL], s1_s[:, :]).then_inc(
                    vch, 1
                )
                v.wait_ge(vch, 2 * t + 2)
                v.tensor_add(hst_s[:, :], n_s[:, :], s2_s[:, :]).then_inc(v2p, 1)

                # move the transposed tile into the broadcast source buffer
                if not no_transp:
                    v.wait_ge(p2v, t + 1)
                    if t >= 2 and not no_bcast:
                        v.wait_ge(lsem[t % 2], 128 * (t // 2))  # t-2 sends done
                    v.tensor_copy(
                        tb_s[:, (t % 2) * P : (t % 2) * P + P], ps_t[t % 2][:, :]
                    ).then_inc(tdone, 1)

                ph1_copy(PRO + t)

            v.wait_ge(psum_rdy, 1 if no_rec else t_steps + 1)
            v.tensor_copy(out_s[:, :], ps_p1f[0:B, 0:SL]).then_inc(fin_sem, 1)

        # ---------------- GPSIMD: remote broadcasts ------------------------
        @block.gpsimd
        def _(g):
            if no_bcast or no_rec:
                return
            # start barrier: no core may broadcast into peers' hbuf until every
            # core has zero-initialized its own hbuf.
            g.wait_ge(hz_sem, 1)
            g.remote_sem_update_broadcast(
                remote_sem=bar_sem,
                local_sem=bar_l,
                rdests=[(0, k) for k in range(N)],
            ).then_inc(bar_p, 1)
            g.wait_ge(bar_p, 1)
            g.trigger_dma(count=1)
            g.wait_ge(bar_sem, 16)
            # barrier passed: every core has zeroed hbuf, so peers' step-0
            # broadcasts may now arrive at any time.

            for t in range(t_steps):
                # order the desc-gen after this step's matmuls: the remote
                # writes must be provably after every receiver's step t-1
                # reads, a chain that runs through our rsem waits.
                g.wait_ge(psum_rdy, t + 1)
                for d in range(N):
                    rd = [None] * N
                    rd[d] = (0, d)
                    g.remote_dma_broadcast(
                        out_ap=hbuf[:, hb(t + 1) + d * P : hb(t + 1) + (d + 1) * P],
                        in_ap=tb_s[:, (t % 2) * P : (t % 2) * P + P],
                        remote_sem=rsem[t % 2][d],
                        local_sem=lsem[t % 2],
                        rdests=rd,
                    ).then_inc(prep_sem, 1)
                g.wait_ge(prep_sem, N * (t + 1))
                g.wait_ge(tdone, t + 1)
                g.trigger_dma(count=N)

    nc.finalize()
    return nc


# ---- host-side input preparation ---------------------------------------------

BF16_NP = ml_dtypes.bfloat16


def _own_rows(r: int) -> np.ndarray:
    """Row indices (into 3H) of core r's r/z/n gate slices."""
    base = np.arange(r * SL, (r + 1) * SL)
    return np.concatenate([base, H + base, 2 * H + base])


def make_core_inputs(
    r,
    xT_shared,
    Wih_f,
    Whh_f,
    bih_f,
    bhh_f,
    Wih_b,
    Whh_b,
    bih_b,
    bhh_b,
    W_lin,
    b_lin,
):
    rows = _own_rows(r)
    perm = [sigma(r, d) for d in range(N)]

    def wih_pack():
        wf = np.ascontiguousarray(Wih_f[rows, :].T)  # [I, 384]
        wb = np.ascontiguousarray(Wih_b[rows, :].T)
        o = np.empty((KT, P, 2 * G3), dtype=BF16_NP)
        for k in range(KT):
            o[k, :, 0:G3] = wf[k * P : (k + 1) * P, :]
            o[k, :, G3 : 2 * G3] = wb[k * P : (k + 1) * P, :]
        return o

    def whh_pack():
        wf = np.ascontiguousarray(Whh_f[rows, :].T)  # [H, 384]
        wb = np.ascontiguousarray(Whh_b[rows, :].T)
        o = np.empty((KT, P, 2 * G3), dtype=BF16_NP)
        for d in range(N):
            s = perm[d]
            o[d, :, 0:G3] = wf[s * P : (s + 1) * P, :]
            o[d, :, G3 : 2 * G3] = wb[s * P : (s + 1) * P, :]
        return o

    def wlin_pack():
        wl = np.ascontiguousarray(W_lin[r * SL : (r + 1) * SL, :].T)  # [2H, 128]
        o = np.empty((2 * KT, P, SL), dtype=BF16_NP)
        for d in range(N):
            s = perm[d]
            o[d] = wl[s * P : (s + 1) * P, :]
            o[N + d] = wl[H + s * P : H + (s + 1) * P, :]
        return o

    brz_f = (bih_f + bhh_f)[rows]
    brz_b = (bih_b + bhh_b)[rows]
    b1 = np.empty((1, 2 * G3), dtype=BF16_NP)
    b1[0, 0 : 2 * SL] = brz_f[0 : 2 * SL]
    b1[0, 2 * SL : G3] = bih_f[rows][2 * SL : G3]
    b1[0, G3 : G3 + 2 * SL] = brz_b[0 : 2 * SL]
    b1[0, G3 + 2 * SL : 2 * G3] = bih_b[rows][2 * SL : G3]

    bn = np.empty((1, 2 * SL), dtype=BF16_NP)
    bn[0, 0:SL] = bhh_f[rows][2 * SL : G3]
    bn[0, SL : 2 * SL] = bhh_b[rows][2 * SL : G3]

    return {
        "xT": xT_shared,
        "wih": wih_pack(),
        "whh": whh_pack(),
        "wlin": wlin_pack(),
        "bias1": b1,
        "biasn": bn,
        "blin": b_lin[r * SL : (r + 1) * SL].reshape(1, SL).astype(BF16_NP),
        "ident": np.eye(P, dtype=BF16_NP),
        "ones": np.ones((1, P), dtype=BF16_NP),
    }


def make_xT(input_btI: np.ndarray, t_steps: int = T) -> np.ndarray:
    """[B,T,I] -> [NTT, P, KT*P] bf16, token order (k, t_off, b) in the free dim."""
    ntt = t_steps // 2
    xt = np.transpose(input_btI, (1, 0, 2))  # [T, B, I]
    v = xt.reshape(ntt, 2, B, KT, P)  # [tau, toff, b, k, i]
    v = np.transpose(v, (0, 4, 3, 1, 2))  # [tau, i, k, toff, b]
    return np.ascontiguousarray(v.reshape(ntt, P, KT * P)).astype(BF16_NP)


_PROG_CACHE: dict = {}

LAST_EXEC_NS = None
LAST_RESULTS = None


def get_program(t_steps: int = T):
    if t_steps not in _PROG_CACHE:
        _PROG_CACHE[t_steps] = build_program(t_steps)
    return _PROG_CACHE[t_steps]


def kernel(
    input,
    Wih_f,
    Whh_f,
    bih_f,
    bhh_f,
    Wih_b,
    Whh_b,
    bih_b,
    bhh_b,
    W_lin,
    b_lin,
):
    from concourse.bass_utils import run_bass_kernel_spmd

    args = [
        np.asarray(a, dtype=np.float32)
        for a in (Wih_f, Whh_f, bih_f, bhh_f, Wih_b, Whh_b, bih_b, bhh_b, W_lin, b_lin)
    ]
    x = np.asarray(input, dtype=np.float32)
    xT_shared = make_xT(x, T)
    nc = get_program(T)
    in_maps = [make_core_inputs(r, xT_shared, *args) for r in range(N)]
    kwargs = {}
    if os.environ.get("KTRACE"):
        kwargs["trace"] = True
        if os.environ.get("KTMPDIR"):
            kwargs["tmpdir"] = os.environ["KTMPDIR"]
        if os.environ.get("KTRACE_CORES"):
            kwargs["trace_cores"] = [
                int(c) for c in os.environ["KTRACE_CORES"].split(",")
            ]
    bk = run_bass_kernel_spmd(nc, in_maps, list(range(N)), **kwargs)
    global LAST_EXEC_NS, LAST_RESULTS
    LAST_EXEC_NS = bk.exec_time_ns
    LAST_RESULTS = bk
    res = bk.results
    out = np.concatenate([res[r]["out"] for r in range(N)], axis=1)
    return np.ascontiguousarray(out).astype(np.float32)



# revision 19
# speedup vs baseline: 3.1295x; 3.1295x over previous
"""BiGRU Trainium2 kernel, 8-core SPMD.

Strategy: shard the hidden dimension H=1024 8 ways (128 per core). Each core
computes its 128-wide slice of both GRU directions for the full batch; the
per-step hidden state is exchanged between all cores with SWDGE remote DMA
(SBUF -> SBUF, one receive slot per peer). The recurrence matmul is
hidden-state-stationary: lhsT = h^T tiles (K = H on partitions), rhs = Whh^T
column slices, so the PE streams weight columns at full rate; fwd and bwd
directions run concurrently on the two halves of the PE array (out partition
base 0 / 64).

The input projection xg = x @ Wih^T + biases (both directions) is computed
on-device, interleaved with the recurrence to fill PE idle time, and staged
through DRAM in [t*B + b] row order so each step loads contiguous tiles.

SPMD twist: remote-DMA relative destinations XOR the *physical* NC index and
instruction streams are identical on all cores, so per-core differences live
in data only. Receive slot d on logical core r holds the h-slice of core
sigma_r(d) = FINV[F[r] ^ d] (F = logical->physical NC map); the host permutes
each core's Whh^T / W_lin^T contraction blocks by sigma_r so one static slot
order is correct everywhere.
"""

import os
import sys

sys.path.insert(0, "/opt/trn_rl_repo")

import numpy as np
import ml_dtypes

import concourse.bass as bass
import concourse.mybir as mybir

# ---- problem constants -------------------------------------------------------
B = 64  # batch
T = 512  # sequence length
I = 1024  # input features
H = 1024  # hidden
O = 1024  # output features
N = 8  # cores
KT = 8  # 128-row contraction blocks in H (and I)
P = 128
SL = 128  # per-core H slice
G3 = 3 * SL  # per-core gate columns (r|z|n)

# logical -> physical NC map of this fabric. Slot routing is XOR-delta based,
# so only deltas matter for the host-side weight permutation; the absolute map
# and the chip routing id are measured at runtime by _probe_fabric() (the
# remote_dma_fused transfers need absolute pid/rid).
F_MAP = [0, 1, 2, 3, 6, 7, 4, 5]
FINV = [F_MAP.index(i) for i in range(8)]

BF16 = mybir.dt.bfloat16
F32 = mybir.dt.float32
AFT = mybir.ActivationFunctionType


def sigma(r: int, d: int) -> int:
    """H-slice owner whose tile lands in receive slot d on logical core r."""
    return FINV[F_MAP[r] ^ d]


# ---- fabric probe ------------------------------------------------------------
# bits 36-47 of a 64-bit fabric address select the TPB base
_TPB_SEL = {0x002: 0, 0x003: 1, 0x006: 2, 0x007: 3,
            0x802: 4, 0x803: 5, 0x806: 6, 0x807: 7}

_FABRIC = None  # (rid, pmap) discovered at first kernel() call


def _build_probe():
    """SPMD probe: each core PREPARE_ONLYs a relative sem-update to ITSELF,
    dumps the SWDGE descriptor ring (SBUF carveout at addr 0), then fires the
    harmless self-update. The rx sem descriptor's address carries the chip
    routing id (bits 48-53, ID_VALID bit 54) and own TPB base (bits 36-47)."""
    from concourse.bacc import Bacc
    from contextlib import ExitStack

    nc = Bacc()
    dbg = nc.declare_dram_parameter("dbg", [128, 2048], mybir.dt.uint8, isOutput=True)
    es = ExitStack()
    with es:
        block = es.enter_context(nc.Block())
        psem = es.enter_context(nc.semaphore("psem"))
        dsem = es.enter_context(nc.semaphore("dsem"))
        rsem = es.enter_context(nc.semaphore("rsem"))
        lsem = es.enter_context(nc.semaphore("lsem"))

        @block.gpsimd
        def _(g):
            g.remote_sem_update_broadcast(
                remote_sem=rsem, local_sem=lsem, rdests=[(0, 0)] + [None] * 7
            ).then_inc(psem, 1)
            g.wait_ge(dsem, 16)
            g.trigger_dma(count=1)
            g.wait_ge(rsem, 2)

        @block.sync
        def _(s):
            s.wait_ge(psem, 1)
            s.dma_start(out=dbg[:, :], in_=nc.dma_scratch[0:128, 0:2048]).then_inc(
                dsem, 16
            )

    nc.finalize()
    return nc


def _parse_probe_dump(dump: np.ndarray):
    words = np.ascontiguousarray(dump.reshape(128, -1)).view(np.uint32)
    found: dict = {}
    for p in range(words.shape[0]):
        row = words[p]
        for k in range(0, row.shape[0] - 3):
            w2, w3 = int(row[k + 2]), int(row[k + 3])
            if not (w3 >> 22) & 1:  # ID_VALID (addr bit 54)
                continue
            sel = (w3 >> 4) & 0xFFF
            if sel not in _TPB_SEL:
                continue
            if (w3 & 0xF) != 0x8 or ((w2 >> 20) & 0xFFF) != 0x027:
                continue
            key = ((w3 >> 16) & 0xF, _TPB_SEL[sel])
            found[key] = found.get(key, 0) + 1
    return found


def _probe_fabric():
    global _FABRIC
    if _FABRIC is not None:
        return _FABRIC
    from concourse.bass_utils import run_bass_kernel_spmd

    nc = _build_probe()
    res = run_bass_kernel_spmd(nc, [{} for _ in range(N)], list(range(N))).results
    rids, pmap = [], []
    for r in range(N):
        cand = _parse_probe_dump(res[r]["dbg"])
        if not cand:
            raise RuntimeError(f"fabric probe: core {r} found no routed descriptor")
        (rid, tpb), _ = max(cand.items(), key=lambda kv: kv[1])
        rids.append(rid)
        pmap.append(tpb)
    assert len(set(rids)) == 1, f"cores on different devices? rids={rids}"
    assert sorted(pmap) == list(range(N)), f"bad physical map {pmap}"
    # the measured map must agree with F_MAP up to a constant XOR (deltas
    # drive the slot convention the host-side weight packing uses)
    x = pmap[0] ^ F_MAP[0]
    assert all(pmap[i] == F_MAP[i] ^ x for i in range(N)), (F_MAP, pmap)
    _FABRIC = (rids[0], pmap)
    return _FABRIC


# ---- device program ----------------------------------------------------------


def build_program(t_steps: int = T):
    """One SPMD Bacc program, identical for all 8 cores.

    t_steps must be even; the phase-1 token tiling assumes
    n_tok_tiles = t_steps / 2 (each tile = 2 t-values x 64 batch rows).
    """
    from concourse.bacc import Bacc

    assert t_steps % 2 == 0
    NTT = t_steps // 2  # phase-1 token tiles
    PRO = min(4, NTT)  # tiles processed before step 0
    XPF = 4  # xg prefetch depth (ring)

    DBG = os.environ.get("KDBG", "").split(",")
    no_bcast = "nobcast" in DBG
    no_epi = "noepi" in DBG
    no_rec = "norec" in DBG
    no_gates = "nogates" in DBG
    no_transp = "notransp" in DBG
    no_mmrec = "nommrec" in DBG
    act_only = "actonly" in DBG
    no_tanh = "notanh" in DBG
    no_dvemix = "nodvemix" in DBG

    nc = Bacc()

    # -- IO -------------------------------------------------------------------
    xT = nc.declare_dram_parameter("xT", [NTT, P, KT * P], BF16, isOutput=False)
    wih = nc.declare_dram_parameter("wih", [KT, P, 2 * G3], BF16, isOutput=False)
    whh = nc.declare_dram_parameter("whh", [KT, P, 2 * G3], BF16, isOutput=False)
    wlin = nc.declare_dram_parameter("wlin", [2 * KT, P, SL], BF16, isOutput=False)
    bias1 = nc.declare_dram_parameter("bias1", [1, 2 * G3], BF16, isOutput=False)
    biasn = nc.declare_dram_parameter("biasn", [1, 2 * SL], BF16, isOutput=False)
    blin = nc.declare_dram_parameter("blin", [1, SL], BF16, isOutput=False)
    ident = nc.declare_dram_parameter("ident", [P, P], BF16, isOutput=False)
    ones = nc.declare_dram_parameter("ones", [1, P], BF16, isOutput=False)
    # per-core peer routing: [pid of slot-d peer for d in 0..7], then rid
    peers = nc.declare_dram_parameter(
        "peers", [1, 16], mybir.dt.uint32, isOutput=False
    )
    out = nc.declare_dram_parameter("out", [B, SL], F32, isOutput=True)

    # phase-1 output staging through DRAM, [t*64 + b, 384] row order
    xgf_d = nc.dram_tensor("xgf_d", [t_steps * B, G3], BF16)
    xgb_d = nc.dram_tensor("xgb_d", [t_steps * B, G3], BF16)

    n_init_dma = KT + KT + 2 * KT + 6  # whh, wih, wlin blocks + 6 small consts

    def ph1_tile(p: int) -> int:
        """phase-1 processing order: ends inward (0, NTT-1, 1, NTT-2, ...)."""
        return p // 2 if p % 2 == 0 else NTT - 1 - p // 2

    from contextlib import ExitStack

    es = ExitStack()
    with es:
        sem = lambda name: es.enter_context(nc.semaphore(name))
        sbuf = lambda name, shape, dt=BF16: es.enter_context(
            nc.sbuf_tensor(name, shape, dt)
        )
        psum = lambda name, shape, dt: es.enter_context(nc.psum_tensor(name, shape, dt))

        block = es.enter_context(nc.Block())
        init_sem = sem("init_sem")
        hz_sem = sem("hz_sem")
        bar_sem = sem("bar_sem")
        bar_p = sem("bar_p")
        bar_l = sem("bar_l")
        rsem = [[sem(f"rsem{par}_{d}") for d in range(N)] for par in range(2)]
        lsem = [sem("lsem0"), sem("lsem1")]
        prep_sem = sem("prep_sem")
        psum_rdy = sem("psum_rdy")
        a2v_r = sem("a2v_r")
        a2v_h = sem("a2v_h")
        a2v_z = sem("a2v_z")
        a2v_n = sem("a2v_n")
        v2a_np = sem("v2a_np")
        pf_v = sem("pf_v")
        v2p = sem("v2p")
        vch = sem("vch")
        p2v = sem("p2v")
        tdone = sem("tdone")
        xg_dma = [sem(f"xg_dma{i}") for i in range(XPF)]
        xgc_p = sem("xgc_p")
        xt_dma = [sem("xt_dma0"), sem("xt_dma1")]
        p1_rdy = sem("p1_rdy")
        p1_cp = sem("p1_cp")
        p1_w = [sem("p1_w0"), sem("p1_w1")]
        fin_sem = sem("fin_sem")

        whh_s = sbuf("whh_s", [P, KT * 2 * G3])
        wih_s = sbuf("wih_s", [P, KT * 2 * G3])
        wlin_s = sbuf("wlin_s", [P, 2 * KT * SL])
        hbuf = sbuf("hbuf", [P, 2 * N * P])
        xg_s = sbuf("xg_s", [P, XPF * G3])
        xt_s = sbuf("xt_s", [P, 2 * KT * P])
        rz_s = sbuf("rz_s", [P, 2 * SL])
        t1_s = sbuf("t1_s", [P, SL])
        npre_s = sbuf("npre_s", [P, SL])
        n_s = sbuf("n_s", [P, SL])
        s1_s = sbuf("s1_s", [P, SL])
        s2_s = sbuf("s2_s", [P, SL])
        hst_s = sbuf("hst_s", [P, SL])
        hgn_s = sbuf("hgn_s", [P, SL])
        tb_s = sbuf("tb_s", [P, 2 * P])
        xgf_st = sbuf("xgf_st", [P, 2 * G3])
        xgb_st = sbuf("xgb_st", [P, 2 * G3])
        ident_s = sbuf("ident_s", [P, P])
        ones_s = sbuf("ones_s", [1, P])
        peers_s = sbuf("peers_s", [1, 16], mybir.dt.uint32)
        bias1_s = sbuf("bias1_s", [1, 2 * G3])
        biasn_s = sbuf("biasn_s", [1, 2 * SL])
        blin_s = sbuf("blin_s", [1, SL])
        out_s = sbuf("out_s", [B, SL], F32)
        # separate tensors so double-buffers land in different PSUM banks
        # (PE-write + DVE-read of one bank is a hardware fault)
        ps_rec0 = psum("ps_rec0", [P, G3], F32)
        ps_rec1 = psum("ps_rec1", [P, G3], F32)
        ps_t0 = psum("ps_t0", [P, P], BF16)
        ps_t1 = psum("ps_t1", [P, P], BF16)
        ps_p1f = psum("ps_p1f", [P, G3], F32)
        ps_p1b = psum("ps_p1b", [P, G3], F32)
        ps_rec = [ps_rec0, ps_rec1]
        ps_t = [ps_t0, ps_t1]

        def hb(t):
            """hbuf column offset of the buffer read at step t."""
            return (t % 2) * N * P

        # ---------------- SYNC: all HWDGE DMA traffic ---------------------
        @block.sync
        def _(s):
            for k in range(KT):
                s.dma_start(
                    out=whh_s[:, k * 2 * G3 : (k + 1) * 2 * G3], in_=whh[k, :, :]
                ).then_inc(init_sem, 16)
                s.dma_start(
                    out=wih_s[:, k * 2 * G3 : (k + 1) * 2 * G3], in_=wih[k, :, :]
                ).then_inc(init_sem, 16)
            for k in range(2 * KT):
                s.dma_start(
                    out=wlin_s[:, k * SL : (k + 1) * SL], in_=wlin[k, :, :]
                ).then_inc(init_sem, 16)
            s.dma_start(out=ident_s[:, :], in_=ident[:, :]).then_inc(init_sem, 16)
            s.dma_start(out=ones_s[:, :], in_=ones[:, :]).then_inc(init_sem, 16)
            s.dma_start(out=peers_s[:, :], in_=peers[:, :]).then_inc(init_sem, 16)
            s.dma_start(out=bias1_s[:, :], in_=bias1[:, :]).then_inc(init_sem, 16)
            s.dma_start(out=biasn_s[:, :], in_=biasn[:, :]).then_inc(init_sem, 16)
            s.dma_start(out=blin_s[:, :], in_=blin[:, :]).then_inc(init_sem, 16)

            def load_xt(p):
                if p >= NTT:
                    return
                if p >= 2:
                    s.wait_ge(p1_rdy, p - 1)  # xt ring slot free
                s.dma_start(
                    out=xt_s[:, (p % 2) * KT * P : ((p % 2) + 1) * KT * P],
                    in_=xT[ph1_tile(p), :, :],
                ).then_inc(xt_dma[p % 2], 16)

            def write_ph1(p):
                if p >= NTT:
                    return
                tau = ph1_tile(p)
                s.wait_ge(p1_cp, 2 * (p + 1))
                s.dma_start(
                    out=xgf_d[2 * tau * B : 2 * tau * B + P, :],
                    in_=xgf_st[:, (p % 2) * G3 : (p % 2) * G3 + G3],
                ).then_inc(p1_w[p % 2], 16)
                s.dma_start(
                    out=xgb_d[2 * tau * B : 2 * tau * B + P, :],
                    in_=xgb_st[:, (p % 2) * G3 : (p % 2) * G3 + G3],
                ).then_inc(p1_w[p % 2], 16)

            def load_xg(t):
                if no_rec or t >= t_steps or t < 0:
                    return
                if load_xg.done >= t + 1:
                    return
                load_xg.done = t + 1
                # phase-1 tiles 0..need_p-1 cover fwd row t and bwd row T-1-t
                need_p = min(2 * (t // 2) + 2, NTT)
                s.wait_ge(p1_w[0], 32 * (need_p - need_p // 2))
                s.wait_ge(p1_w[1], 32 * (need_p // 2))
                if t >= XPF:
                    s.wait_ge(v2a_np, t - XPF + 1)
                    s.wait_ge(xgc_p, t - XPF + 1)
                slot = (t % XPF) * G3
                s.dma_start(
                    out=xg_s[0:B, slot : slot + G3],
                    in_=xgf_d[t * B : (t + 1) * B, :],
                ).then_inc(xg_dma[t % XPF], 16)
                s.dma_start(
                    out=xg_s[B:P, slot : slot + G3],
                    in_=xgb_d[(t_steps - 1 - t) * B : (t_steps - t) * B, :],
                ).then_inc(xg_dma[t % XPF], 16)

            # prologue: interleave so no FIFO head-of-line cycle forms
            # (load_xt(p+2) transitively needs write_ph1(p-2) through PE/DVE)
            for p in range(4):
                load_xt(p)
            write_ph1(0)
            load_xt(4)
            write_ph1(1)
            load_xt(5)
            write_ph1(2)
            write_ph1(3)
            load_xg.done = 0
            for t in range(XPF):
                load_xg(t)
            for t in range(t_steps):
                write_ph1(PRO + t)
                load_xt(PRO + t + 2)
                load_xg(t + XPF - 1)

            s.wait_ge(fin_sem, 1)
            s.dma_start(out=out[:, :], in_=out_s[:, :]).then_inc(fin_sem, 16)

        # ---------------- PE: matmuls, transpose, phase-1 ------------------
        @block.tensor
        def _(pe):
            def ph1_work(p):
                if p >= NTT:
                    return
                pe.wait_ge(xt_dma[p % 2], 16 * (p // 2 + 1))
                if p >= 1:
                    pe.wait_ge(p1_cp, 2 * p)  # psum consumed by DVE copies
                xo = (p % 2) * KT * P
                for k in range(KT):
                    lt = xt_s[:, xo + k * P : xo + (k + 1) * P]
                    pe.matmul(
                        ps_p1f[:, :],
                        lt,
                        wih_s[:, k * 2 * G3 : k * 2 * G3 + G3],
                        start=(k == 0),
                        stop=False,
                    )
                    pe.matmul(
                        ps_p1b[:, :],
                        lt,
                        wih_s[:, k * 2 * G3 + G3 : (k + 1) * 2 * G3],
                        start=(k == 0),
                        stop=False,
                    )
                pe.matmul(
                    ps_p1f[:, :],
                    ones_s[0:1, :],
                    bias1_s[0:1, 0:G3],
                    start=False,
                    stop=True,
                )
                pe.matmul(
                    ps_p1b[:, :],
                    ones_s[0:1, :],
                    bias1_s[0:1, G3 : 2 * G3],
                    start=False,
                    stop=True,
                ).then_inc(p1_rdy, 1)

            pe.wait_ge(init_sem, 16 * n_init_dma)
            pe.wait_ge(hz_sem, 2)
            for p in range(PRO):
                ph1_work(p)

            for t in range(t_steps):
                ps = ps_rec[t % 2]
                if no_rec:
                    ph1_work(PRO + t)
                    continue
                if t >= 2:
                    pe.wait_ge(a2v_z, t - 1)
                    pe.wait_ge(pf_v, t - 1)
                pe.wait_ge(xg_dma[t % XPF], 32 * (t // XPF + 1))
                hbo = hb(t)
                slot = (t % XPF) * G3
                if no_mmrec:
                    pe.matmul(
                        ps[:, 0 : 2 * SL],
                        ident_s[:, :],
                        xg_s[:, slot : slot + 2 * SL],
                        start=True,
                        stop=True,
                    ).then_inc(xgc_p, 1)
                    pe.matmul(
                        ps[0:B, 2 * SL : G3],
                        ones_s[0:1, 0:B],
                        biasn_s[0:1, 0:SL],
                        start=True,
                        stop=True,
                    )
                    pe.matmul(
                        ps[B:P, 2 * SL : G3],
                        ones_s[0:1, B:P],
                        biasn_s[0:1, SL : 2 * SL],
                        start=True,
                        stop=True,
                        skip_group_check=True,
                    ).then_inc(psum_rdy, 1)
                if not no_mmrec:
                    # d=0 opens the accumulation (start marks the whole bank
                    # pending-zero per partition half); d=7 closes it; the xg and
                    # bhh_n matmuls then accumulate on top (group-check skipped:
                    # their region is a slice of the already-opened groups).
                    # Per-slot rsem waits: matmul d starts as soon as slot d's
                    # tile has arrived, overlapping the broadcast tail.
                    for d in range(N):
                        if t >= 1 and not no_bcast:
                            pe.wait_ge(rsem[(t - 1) % 2][d], 4 * ((t - 1) // 2 + 1))
                        pe.matmul(
                            ps[0:B, :],
                            hbuf[:, hbo + d * P : hbo + d * P + B],
                            whh_s[:, d * 2 * G3 : d * 2 * G3 + G3],
                            start=(d == 0),
                            stop=(d == N - 1),
                        )
                        pe.matmul(
                            ps[B:P, :],
                            hbuf[:, hbo + d * P + B : hbo + (d + 1) * P],
                            whh_s[:, d * 2 * G3 + G3 : (d + 1) * 2 * G3],
                            start=(d == 0),
                            stop=(d == N - 1),
                            skip_group_check=True,
                        )
                    pe.matmul(
                        ps[:, 0 : 2 * SL],
                        ident_s[:, :],
                        xg_s[:, slot : slot + 2 * SL],
                        start=False,
                        stop=False,
                        skip_group_check=True,
                    ).then_inc(xgc_p, 1)
                    pe.matmul(
                        ps[0:B, 2 * SL : G3],
                        ones_s[0:1, 0:B],
                        biasn_s[0:1, 0:SL],
                        start=False,
                        stop=False,
                        skip_group_check=True,
                    )
                    pe.matmul(
                        ps[B:P, 2 * SL : G3],
                        ones_s[0:1, B:P],
                        biasn_s[0:1, SL : 2 * SL],
                        start=False,
                        stop=False,
                        skip_group_check=True,
                    ).then_inc(psum_rdy, 1)

                # phase-1 fill while the gates run on ACT/DVE
                ph1_work(PRO + t)

                # transpose h_new into the broadcast source layout
                if not no_transp:
                    pe.wait_ge(v2p, t + 1)
                    pe.transpose(ps_t[t % 2][:, :], hst_s[:, :], ident_s[:, :]).then_inc(
                        p2v, 1
                    )

            if no_epi or no_rec:
                pe.wait_ge(p1_cp, 2 * NTT)
                pe.matmul(
                    ps_p1f[0:B, 0:SL],
                    ident_s[:, 0:B],
                    blin_s[0:1, :] if False else wlin_s[0:128, 0:SL],
                    start=True,
                    stop=True,
                ).then_inc(psum_rdy, 1 if no_rec else t_steps + 1)
                return

            # final linear: out = [h_fwd | h_bwd] @ W_lin^T + b_lin
            if not no_bcast:
                for d in range(N):
                    pe.wait_ge(
                        rsem[(t_steps - 1) % 2][d], 4 * ((t_steps - 1) // 2 + 1)
                    )
            pe.wait_ge(p1_cp, 2 * NTT)  # ps_p1f free
            hbo = hb(t_steps)
            for d in range(N):
                pe.matmul(
                    ps_p1f[0:B, 0:SL],
                    hbuf[:, hbo + d * P : hbo + d * P + B],
                    wlin_s[:, d * SL : (d + 1) * SL],
                    start=(d == 0),
                    stop=False,
                )
            for d in range(N):
                pe.matmul(
                    ps_p1f[0:B, 0:SL],
                    hbuf[:, hbo + d * P + B : hbo + (d + 1) * P],
                    wlin_s[:, (N + d) * SL : (N + d + 1) * SL],
                    start=False,
                    stop=False,
                )
            pe.matmul(
                ps_p1f[0:B, 0:SL],
                ones_s[0:1, 0:B],
                blin_s[0:1, :],
                start=False,
                stop=True,
            ).then_inc(psum_rdy, 1)

        # ---------------- ACT: sigmoids + tanh ----------------------------
        @block.scalar
        def _(a):
            if no_rec or no_gates:
                return
            for t in range(t_steps):
                ps = ps_rec[t % 2]
                a.wait_ge(psum_rdy, t + 1)
                # r first, hgn second (both feed the n-gate chain), z last
                # (only needed after tanh for the h_new blend)
                a.activation(rz_s[:, 0:SL], ps[:, 0:SL], AFT.Sigmoid).then_inc(
                    a2v_r, 1
                )
                a.activation(hgn_s[:, :], ps[:, 2 * SL : G3], AFT.Copy).then_inc(
                    a2v_h, 1
                )
                a.activation(
                    rz_s[:, SL : 2 * SL], ps[:, SL : 2 * SL], AFT.Sigmoid
                ).then_inc(a2v_z, 1)
                if act_only or no_tanh:
                    a.activation(n_s[:, :], npre_s[:, :], AFT.Sigmoid).then_inc(
                        a2v_n, 1
                    )
                else:
                    a.wait_ge(v2a_np, t + 1)
                    a.activation(n_s[:, :], npre_s[:, :], AFT.Tanh).then_inc(a2v_n, 1)

        # ---------------- DVE: gate arithmetic, copies --------------------
        @block.vector
        def _(v):
            v.memset(hbuf[:, :], 0.0).then_inc(hz_sem, 1)
            v.memset(hst_s[:, :], 0.0).then_inc(hz_sem, 1)
            v.wait_ge(hz_sem, 2)

            def ph1_copy(p):
                if p >= NTT:
                    return
                v.wait_ge(p1_rdy, p + 1)
                if p >= 2:
                    v.wait_ge(p1_w[p % 2], 32 * (p // 2))  # staging slot free
                v.tensor_copy(
                    xgf_st[:, (p % 2) * G3 : (p % 2) * G3 + G3], ps_p1f[:, :]
                )
                v.tensor_copy(
                    xgb_st[:, (p % 2) * G3 : (p % 2) * G3 + G3], ps_p1b[:, :]
                ).then_inc(p1_cp, 2)

            for p in range(PRO):
                ph1_copy(p)

            for t in range(t_steps):
                if no_rec:
                    ph1_copy(PRO + t)
                    continue
                ps = ps_rec[t % 2]
                slot = (t % XPF) * G3
                if no_gates:
                    v.wait_ge(psum_rdy, t + 1)
                    v.tensor_copy(s1_s[:, :], hst_s[:, :]).then_inc(v2p, 1)
                    if not no_transp:
                        v.wait_ge(p2v, t + 1)
                        v.tensor_copy(
                            tb_s[:, (t % 2) * P : (t % 2) * P + P], ps_t[t % 2][:, :]
                        ).then_inc(tdone, 1)
                    ph1_copy(PRO + t)
                    continue
                v.wait_ge(a2v_r, t + 1)
                if act_only:
                    v.tensor_copy(s1_s[:, :], hst_s[:, :]).then_inc(v2p, 1)
                    if not no_transp:
                        v.wait_ge(p2v, t + 1)
                        v.tensor_copy(
                            tb_s[:, (t % 2) * P : (t % 2) * P + P], ps_t[t % 2][:, :]
                        ).then_inc(tdone, 1)
                    ph1_copy(PRO + t)
                    continue
                # t1 = r * hg_n (hg_n staged through SBUF by ACT: a DVE
                # TensorTensor read of PSUM hard-faults this device)
                v.wait_ge(a2v_h, t + 1)
                v.tensor_mul(t1_s[:, :], rz_s[:, 0:SL], hgn_s[:, :]).then_inc(
                    pf_v, 1
                )
                # n_pre = t1 + xg_n
                v.wait_ge(pf_v, t + 1)  # t1 writeback drained
                v.wait_ge(xg_dma[t % XPF], 32 * (t // XPF + 1))
                v.tensor_add(
                    npre_s[:, :], t1_s[:, :], xg_s[:, slot + 2 * SL : slot + G3]
                ).then_inc(v2a_np, 1)
                v.wait_ge(a2v_n, t + 1)
                # h_new = n + z*(h - n)
                if t >= 1:
                    v.wait_ge(v2p, t)  # prior h_new writeback drained
                v.tensor_sub(s1_s[:, :], hst_s[:, :], n_s[:, :]).then_inc(vch, 1)
                v.wait_ge(a2v_z, t + 1)
                v.wait_ge(vch, 2 * t + 1)
                v.tensor_mul(s2_s[:, :], rz_s[:, SL : 2 * SL], s1_s[:, :]).then_inc(
                    vch, 1
                )
                v.wait_ge(vch, 2 * t + 2)
                v.tensor_add(hst_s[:, :], n_s[:, :], s2_s[:, :]).then_inc(v2p, 1)

                # move the transposed tile into the broadcast source buffer
                if not no_transp:
                    v.wait_ge(p2v, t + 1)
                    if t >= 2 and not no_bcast:
                        v.wait_ge(lsem[t % 2], 32 * (t // 2))  # t-2 sends done
                    v.tensor_copy(
                        tb_s[:, (t % 2) * P : (t % 2) * P + P], ps_t[t % 2][:, :]
                    ).then_inc(tdone, 1)

                ph1_copy(PRO + t)

            v.wait_ge(psum_rdy, 1 if no_rec else t_steps + 1)
            v.tensor_copy(out_s[:, :], ps_p1f[0:B, 0:SL]).then_inc(fin_sem, 1)

        # ---------------- GPSIMD: remote h exchange ------------------------
        # 2x remote_dma_fused per step (4 nibble-masked transfers each, zero
        # dummy descriptors), desc-gen pipelined one step ahead; routing is
        # absolute (pid/rid from the per-core peers table via GPRs).
        @block.gpsimd
        def _(g):
            if no_bcast or no_rec:
                return
            from concourse.bass import RemoteDMATransfer

            g.wait_ge(init_sem, 16 * n_init_dma)
            pid_regs = [g.alloc_register(f"pidreg{d}") for d in range(N)]
            rid_reg = g.alloc_register("ridreg")
            for d in range(N):
                g.reg_load(pid_regs[d], peers_s[0:1, d : d + 1])
            g.reg_load(rid_reg, peers_s[0:1, N : N + 1])

            # start barrier: no core may write into peers' hbuf until every
            # core has zero-initialized its own hbuf.
            g.wait_ge(hz_sem, 1)
            g.remote_sem_update_broadcast(
                remote_sem=bar_sem,
                local_sem=bar_l,
                rdests=[(0, k) for k in range(N)],
            ).then_inc(bar_p, 1)
            g.wait_ge(bar_p, 1)
            g.trigger_dma(count=1)
            g.wait_ge(bar_sem, 16)
            # barrier passed: every core has zeroed hbuf, so peers' step-0
            # writes may now arrive at any time.

            # slots 0-3 are same-die (delta bit 2 clear), 4-7 cross-die and
            # must ride D2D-capable engines (4-7, 12-15 = nibbles 1 and 3).
            HALVES = ((0, 1, 4, 5), (2, 3, 6, 7))
            MASKS = (0x000F, 0x0F00, 0x00F0, 0xF000)

            def prep(tt):
                par_t = tt % 2
                hbo_t = hb(tt + 1)
                src = tb_s[:, par_t * P : par_t * P + P]
                for half in HALVES:
                    g.remote_dma_fused(
                        [
                            RemoteDMATransfer(
                                pid=pid_regs[d],
                                routing_id=rid_reg,
                                dma_engine_mask=MASKS[j],
                                remote_sem=rsem[par_t][d],
                                src=src,
                                dst=hbuf[:, hbo_t + d * P : hbo_t + (d + 1) * P],
                            )
                            for j, d in enumerate(half)
                        ],
                        local_sem=lsem[par_t],
                    ).then_inc(prep_sem, 1)

            prep(0)
            for t in range(t_steps):
                if t + 1 < t_steps:
                    prep(t + 1)  # desc-gen one step ahead (addresses only)
                g.wait_ge(prep_sem, 2 * (t + 1))
                # trigger ordering: the remote writes must be provably after
                # every receiver's step t-1 reads, a chain that runs through
                # our rsem waits (see psum_rdy analysis).
                g.wait_ge(psum_rdy, t + 1)
                g.wait_ge(tdone, t + 1)
                g.trigger_dma(count=2)

    nc.finalize()
    return nc


# ---- host-side input preparation ---------------------------------------------

BF16_NP = ml_dtypes.bfloat16


def _own_rows(r: int) -> np.ndarray:
    """Row indices (into 3H) of core r's r/z/n gate slices."""
    base = np.arange(r * SL, (r + 1) * SL)
    return np.concatenate([base, H + base, 2 * H + base])


def make_core_inputs(
    r,
    xT_shared,
    Wih_f,
    Whh_f,
    bih_f,
    bhh_f,
    Wih_b,
    Whh_b,
    bih_b,
    bhh_b,
    W_lin,
    b_lin,
):
    rows = _own_rows(r)
    perm = [sigma(r, d) for d in range(N)]

    def wih_pack():
        wf = np.ascontiguousarray(Wih_f[rows, :].T)  # [I, 384]
        wb = np.ascontiguousarray(Wih_b[rows, :].T)
        o = np.empty((KT, P, 2 * G3), dtype=BF16_NP)
        for k in range(KT):
            o[k, :, 0:G3] = wf[k * P : (k + 1) * P, :]
            o[k, :, G3 : 2 * G3] = wb[k * P : (k + 1) * P, :]
        return o

    def whh_pack():
        wf = np.ascontiguousarray(Whh_f[rows, :].T)  # [H, 384]
        wb = np.ascontiguousarray(Whh_b[rows, :].T)
        o = np.empty((KT, P, 2 * G3), dtype=BF16_NP)
        for d in range(N):
            s = perm[d]
            o[d, :, 0:G3] = wf[s * P : (s + 1) * P, :]
            o[d, :, G3 : 2 * G3] = wb[s * P : (s + 1) * P, :]
        return o

    def wlin_pack():
        wl = np.ascontiguousarray(W_lin[r * SL : (r + 1) * SL, :].T)  # [2H, 128]
        o = np.empty((2 * KT, P, SL), dtype=BF16_NP)
        for d in range(N):
            s = perm[d]
            o[d] = wl[s * P : (s + 1) * P, :]
            o[N + d] = wl[H + s * P : H + (s + 1) * P, :]
        return o

    brz_f = (bih_f + bhh_f)[rows]
    brz_b = (bih_b + bhh_b)[rows]
    b1 = np.empty((1, 2 * G3), dtype=BF16_NP)
    b1[0, 0 : 2 * SL] = brz_f[0 : 2 * SL]
    b1[0, 2 * SL : G3] = bih_f[rows][2 * SL : G3]
    b1[0, G3 : G3 + 2 * SL] = brz_b[0 : 2 * SL]
    b1[0, G3 + 2 * SL : 2 * G3] = bih_b[rows][2 * SL : G3]

    bn = np.empty((1, 2 * SL), dtype=BF16_NP)
    bn[0, 0:SL] = bhh_f[rows][2 * SL : G3]
    bn[0, SL : 2 * SL] = bhh_b[rows][2 * SL : G3]

    rid, pmap = _FABRIC
    pt = np.zeros((1, 16), dtype=np.uint32)
    for d in range(N):
        pt[0, d] = pmap[r] ^ d  # physical NC of the slot-d peer
    pt[0, N] = rid

    return {
        "xT": xT_shared,
        "wih": wih_pack(),
        "whh": whh_pack(),
        "wlin": wlin_pack(),
        "bias1": b1,
        "biasn": bn,
        "blin": b_lin[r * SL : (r + 1) * SL].reshape(1, SL).astype(BF16_NP),
        "ident": np.eye(P, dtype=BF16_NP),
        "ones": np.ones((1, P), dtype=BF16_NP),
        "peers": pt,
    }


def make_xT(input_btI: np.ndarray, t_steps: int = T) -> np.ndarray:
    """[B,T,I] -> [NTT, P, KT*P] bf16, token order (k, t_off, b) in the free dim."""
    ntt = t_steps // 2
    xt = np.transpose(input_btI, (1, 0, 2))  # [T, B, I]
    v = xt.reshape(ntt, 2, B, KT, P)  # [tau, toff, b, k, i]
    v = np.transpose(v, (0, 4, 3, 1, 2))  # [tau, i, k, toff, b]
    return np.ascontiguousarray(v.reshape(ntt, P, KT * P)).astype(BF16_NP)


_PROG_CACHE: dict = {}

LAST_EXEC_NS = None
LAST_RESULTS = None


def get_program(t_steps: int = T):
    if t_steps not in _PROG_CACHE:
        _PROG_CACHE[t_steps] = build_program(t_steps)
    return _PROG_CACHE[t_steps]


def kernel(
    input,
    Wih_f,
    Whh_f,
    bih_f,
    bhh_f,
    Wih_b,
    Whh_b,
    bih_b,
    bhh_b,
    W_lin,
    b_lin,
):
    from concourse.bass_utils import run_bass_kernel_spmd

    _probe_fabric()
    args = [
        np.asarray(a, dtype=np.float32)
        for a in (Wih_f, Whh_f, bih_f, bhh_f, Wih_b, Whh_b, bih_b, bhh_b, W_lin, b_lin)
    ]
    x = np.asarray(input, dtype=np.float32)
    t_steps = x.shape[1]
    xT_shared = make_xT(x, t_steps)
    nc = get_program(t_steps)
    in_maps = [make_core_inputs(r, xT_shared, *args) for r in range(N)]
    kwargs = {}
    if os.environ.get("KTRACE"):
        kwargs["trace"] = True
        if os.environ.get("KTMPDIR"):
            kwargs["tmpdir"] = os.environ["KTMPDIR"]
        if os.environ.get("KTRACE_CORES"):
            kwargs["trace_cores"] = [
                int(c) for c in os.environ["KTRACE_CORES"].split(",")
            ]
    bk = run_bass_kernel_spmd(nc, in_maps, list(range(N)), **kwargs)
    global LAST_EXEC_NS, LAST_RESULTS
    LAST_EXEC_NS = bk.exec_time_ns
    LAST_RESULTS = bk
    res = bk.results
    out = np.concatenate([res[r]["out"] for r in range(N)], axis=1)
    return np.ascontiguousarray(out).astype(np.float32)

